# revision 6
# baseline (speedup 1.0000x reference)
"""Trainium2 Bass kernel for FlowNet-style Correlation (MAX_DISP=4).

Input:  x_1, x_2  [8, 64, 256, 256] f32
Output:           [8, 81, 256, 256] f32
out[b, 9*dx+dy, h, w] = mean_c x1[b,c,h,w] * x2pad[b,c,h+dx,w+dy]
(x2pad zero-padded by 4 on each spatial side)

Strategy (data-parallel, 1 image per NeuronCore, 8 cores):
- H-strip band matmuls with BLOCK-DIAGONAL band pairing: the stationary
  [128, 64] packs two strips (same w column, band k rows 0-31 on contraction
  rows 0-63, band k+1 on rows 64-127) so one fp32r matmul (1 cycle/row at
  moving>=256) computes both strips in a single 360-column pass.
- x2 window lives in SBUF as [128, 40, 264]: partitions 0-63 hold rows
  [h0-4, h0+36), partitions 64-127 hold rows [h0+28, h0+68).
- psum[p = 64*wp + 32*band + i, 9r+dy]; useful run per pixel i is the
  contiguous psum[p, 9i : 9i+81] (diagonal across partitions -> not
  extractable on-chip with uniform APs).
- Copy psum -> SBUF in bf16 (vector/scalar/gpsimd rotate), dump to DRAM,
  re-read the diagonal with a DRAM-side strided AP (batched: one DMA per
  (band, half, i-group)), PE-transpose in bf16, and write bf16 output rows
  (host casts back to f32).
"""
import os
import numpy as np
from contextlib import ExitStack

import concourse.bass as bass
import concourse.tile as tile
from concourse import bacc, mybir
from concourse.bass_utils import run_bass_kernel_spmd

F32 = mybir.dt.float32
F32R = mybir.dt.float32r
BF16 = mybir.dt.bfloat16

B, C, H, W = 8, 64, 256, 256
MD = 4
D = 2 * MD + 1          # 9
DD = D * D              # 81
BAND = 32               # strip height
NDB = 4                 # double-bands (64 rows each)
WIN = 40                # x2 window rows per band
NCOL = WIN * D          # 360 psum columns
NT = 128                # psum tiles per double-band (2 pairs x 2 bands each)
TILE_ELEMS = 128 * NCOL  # dump elements per psum tile

USE_POOL_STG = True     # rotate gpsimd into the psum->sbuf copies


def _ap(t, p0, np_, free_dims, free_off=0):
    """Custom AP over a tile: partitions [p0, p0+np_), free dims in flat
    elements of the tile's free extent."""
    base = t[:]
    pitch = base.ap[0][0]
    return bass.AP(
        tensor=t.tensor,
        offset=base.offset + p0 * pitch + free_off,
        ap=[[pitch, np_]] + free_dims,
    )


def build_kernel():
    nc = bacc.Bacc("TRN2", target_bir_lowering=False, debug=False)
    x1 = nc.dram_tensor("x1", [C, H, W], F32, kind="ExternalInput").ap()
    x2 = nc.dram_tensor("x2", [C, H, W], F32, kind="ExternalInput").ap()
    ident = nc.dram_tensor("ident", [128, 128], F32, kind="ExternalInput").ap()
    out = nc.dram_tensor("out", [DD, H, W], BF16, kind="ExternalOutput").ap()

    with tile.TileContext(nc) as tc, ExitStack() as ctx:
        x1pool = ctx.enter_context(tc.tile_pool(name="x1pool", bufs=2))
        x2pool = ctx.enter_context(tc.tile_pool(name="x2pool", bufs=2))
        stgp = ctx.enter_context(tc.tile_pool(name="stgp", bufs=3))
        dgp = ctx.enter_context(tc.tile_pool(name="dgp", bufs=3))
        osp = ctx.enter_context(tc.tile_pool(name="osp", bufs=2))
        cpool = ctx.enter_context(tc.tile_pool(name="cpool", bufs=1))
        pspool = ctx.enter_context(tc.tile_pool(name="pspool", bufs=3, space="PSUM"))
        tpsp = ctx.enter_context(tc.tile_pool(name="tpsp", bufs=2, space="PSUM"))
        drampool = ctx.enter_context(tc.tile_pool(name="drampool", bufs=2, space="DRAM"))

        identf = cpool.tile([128, 128], F32)
        nc.sync.dma_start(out=identf[:], in_=ident)
        identb = cpool.tile([128, 128], BF16)
        nc.vector.tensor_copy(identb[:], identf[:])

        # persistent double-buffered stationaries: zeros off the block
        # diagonal are written once and reused for all quarters
        s_a = cpool.tile([128, 64 * 64], F32, tag="s_a")
        s_b = cpool.tile([128, 64 * 64], F32, tag="s_b")
        s_tiles = [s_a, s_b]
        for st in s_tiles:
            nc.gpsimd.memset(st[:], 0.0)

        tctr = 0   # global psum-tile counter (engine rotation)
        octr = 0   # outstage-copy counter

        for dp in range(NDB):
            h0 = 64 * dp

            # --- x2 window [128, 40, 264]: band a rows [h0-4,h0+36) on
            # partitions 0-63, band b rows [h0+28,h0+68) on 64-127 ---
            x2d = x2pool.tile([128, WIN, W + 8], F32, tag="x2d")
            nc.vector.memset(x2d[:, :, 0:MD], 0.0)
            nc.vector.memset(x2d[:, :, W + MD : W + 8], 0.0)
            r0a = MD if dp == 0 else 0
            if dp == 0:
                nc.gpsimd.memset(x2d[0:64, 0:MD, :], 0.0)
            nc.sync.dma_start(
                out=x2d[0:64, r0a:WIN, MD : MD + W],
                in_=x2[:, h0 - MD + r0a : h0 + 36, :],
            )
            r1b = 36 if dp == NDB - 1 else WIN
            if dp == NDB - 1:
                nc.gpsimd.memset(x2d[64:128, 36:WIN, :], 0.0)
            nc.sync.dma_start(
                out=x2d[64:128, 0:r1b, MD : MD + W],
                in_=x2[:, h0 + 28 : h0 + 28 + r1b, :],
            )

            dump_t = drampool.tile([NT, 128, NCOL], BF16, tag="dump")
            dump_base = dump_t[:].offset

            tile_idx = 0
            stg = None
            for q in range(4):
                w0 = 64 * q
                # x1 quarter: band a rows on partitions 0-63, band b on 64-127
                x1q = x1pool.tile([128, BAND, 64], F32, tag="x1q")
                nc.sync.dma_start(
                    out=x1q[0:64], in_=x1[:, h0 : h0 + 32, w0 : w0 + 64]
                )
                nc.sync.dma_start(
                    out=x1q[64:128], in_=x1[:, h0 + 32 : h0 + 64, w0 : w0 + 64]
                )

                # stationaries: S[:, 64*w' + (0:32)] = band-a strip w' (rows
                # 0-63), S[:, 64*w' + (32:64)] = band-b strip (rows 64-127)
                S = s_tiles[(4 * dp + q) % 2]
                nc.vector.tensor_copy(
                    _ap(S, 0, 64, [[64, 64], [1, 32]], 0),
                    _ap(x1q, 0, 64, [[1, 64], [64, 32]], 0),
                )
                nc.scalar.copy(
                    _ap(S, 64, 64, [[64, 64], [1, 32]], 32),
                    _ap(x1q, 64, 64, [[1, 64], [64, 32]], 0),
                )

                for tq in range(32):
                    wg = w0 + 2 * tq  # global w of even pair
                    ps = pspool.tile([128, 512], F32, tag="ps")
                    nc.tensor.matmul(
                        ps[0:64, 0:NCOL],
                        S[:, 128 * tq : 128 * tq + 64].bitcast(F32R),
                        x2d[:, :, wg : wg + D].bitcast(F32R),
                        start=True, stop=True, tile_position=(0, 0),
                    )
                    nc.tensor.matmul(
                        ps[64:128, 0:NCOL],
                        S[:, 128 * tq + 64 : 128 * tq + 128].bitcast(F32R),
                        x2d[:, :, wg + 1 : wg + D + 1].bitcast(F32R),
                        start=True, stop=True, tile_position=(0, 64),
                    )
                    slot = tile_idx % 4
                    if slot == 0:
                        stg = stgp.tile([128, 4 * NCOL], BF16, tag="stg")
                    eng = tctr % (3 if USE_POOL_STG else 2)
                    dst = stg[:, NCOL * slot : NCOL * (slot + 1)]
                    src = ps[:, 0:NCOL]
                    if eng == 0:
                        nc.vector.tensor_copy(dst, src)
                    elif eng == 1:
                        nc.scalar.copy(dst, src)
                    else:
                        nc.gpsimd.tensor_copy(dst, src)
                    tctr += 1
                    if slot == 3:
                        nc.sync.dma_start(
                            out=bass.AP(
                                tensor=dump_t.tensor,
                                offset=dump_base + (tile_idx - 3) * TILE_ELEMS,
                                ap=[[NCOL, 128], [TILE_ELEMS, 4], [1, NCOL]],
                            ),
                            in_=stg[:],
                        )
                    tile_idx += 1

            # --- extraction: diagonal re-read + bf16 transpose + store ---
            for band in range(2):
                for g in range(2):
                    ostg = osp.tile([DD, 16 * 256], BF16, tag="ostg")
                    for half in range(2):
                        dg = dgp.tile([128, 16, DD], BF16, tag="dg")
                        nc.sync.dma_start(
                            out=dg[:],
                            in_=bass.AP(
                                tensor=dump_t.tensor,
                                offset=dump_base
                                + half * 64 * TILE_ELEMS
                                + band * 32 * NCOL
                                + g * 16 * (NCOL + D),
                                ap=[
                                    [TILE_ELEMS, 64],
                                    [64 * NCOL, 2],
                                    [NCOL + D, 16],
                                    [1, DD],
                                ],
                            ),
                        )
                        for j in range(4):
                            pst = tpsp.tile([DD, 512], BF16, tag="pst")
                            for jj in range(4):
                                nc.tensor.transpose(
                                    pst[:, 128 * jj : 128 * (jj + 1)],
                                    dg[:, 4 * j + jj, :],
                                    identb[:],
                                )
                            dst = _ap(
                                ostg, 0, DD, [[256, 4], [1, 128]],
                                (4 * j) * 256 + 128 * half,
                            )
                            if octr % 2 == 0:
                                nc.vector.tensor_copy(dst, pst[:])
                            else:
                                nc.scalar.copy(dst, pst[:])
                            octr += 1
                    nc.sync.dma_start(
                        out=bass.AP(
                            tensor=out.tensor,
                            offset=(h0 + 32 * band + 16 * g) * W,
                            ap=[[H * W, DD], [W, 16], [1, 256]],
                        ),
                        in_=ostg[:],
                    )

    nc.compile()
    return nc


_NC_CACHE = {}


def _get_nc():
    if "nc" not in _NC_CACHE:
        _NC_CACHE["nc"] = build_kernel()
    return _NC_CACHE["nc"]


def kernel(x_1: np.ndarray, x_2: np.ndarray) -> np.ndarray:
    x_1 = np.asarray(x_1, dtype=np.float32) * np.float32(1.0 / C)
    x_2 = np.asarray(x_2, dtype=np.float32)
    nc = _get_nc()
    eye = np.eye(128, dtype=np.float32)
    in_maps = [
        {"x1": np.ascontiguousarray(x_1[b]), "x2": np.ascontiguousarray(x_2[b]),
         "ident": eye}
        for b in range(B)
    ]
    res = run_bass_kernel_spmd(
        nc, in_maps, core_ids=list(range(B)),
        trace=bool(int(os.environ.get("CORR_TRACE", "0"))),
    )
    outs = [np.asarray(res.results[b]["out"]).astype(np.float32) for b in range(B)]
    if int(os.environ.get("CORR_TRACE", "0")):
        _NC_CACHE["last_results"] = res
    return np.stack(outs, axis=0)
